# revision 26
# baseline (speedup 1.0000x reference)
"""Single-head causal attention (B=16, T=1024, C=768, H=64) on 8 TRN2 cores.

Data-parallel over batch: 2 batch elements per core, weights replicated.
All matmuls run in bf16 (fp32 PSUM accumulation) — 4x PE throughput over
fp32 and half the input DMA traffic. Tolerance is 2e-2; bf16 rounding of
q/k/v/E contributes ~0.5% relative error.

Per batch element, on device:
  qk^T[128, T]  = [Wq|Wk]^T @ x^T   (q on partitions 0..63, k on 64..127)
  v^T[64, T]    = Wv^T @ x^T ; PE-transposed per 128-block into vaug[s, 1+H]
                  with col 0 = ones (gives the softmax denominator for free;
                  v-bias folded in via a precomputed broadcast tile)
  S^T[s,t]      = k^T-block @ q^T (contraction over h); causal handled
                  block-wise: skip all-invalid blocks; on diagonal blocks a
                  second accumulating matmul adds -1e9 to the s>t strip so
                  exp() zeroes it — no vector-engine masking needed.
  E             = exp(scale * S^T)  (ScalarE, bf16 out; logits are O(6), no
                  max-subtraction needed)
  out_aug^T     = vaug^T @ E        (row 0 = denominator, rows 1..64 = num)
  normalize     = den -> bf16 -> PE partition-broadcast -> DVE reciprocal
                  -> DVE multiply; DMA out^T[h, t] per batch.
"""

import numpy as np
from contextlib import ExitStack

import concourse.bass as bass
import concourse.tile as tile
from concourse import mybir
from concourse.vector_clock import ScopedClock

f32 = mybir.dt.float32
bf16 = mybir.dt.bfloat16
AF = mybir.ActivationFunctionType

B, T, C, H = 16, 1024, 768, 64
NCORES = 8
BPC = B // NCORES          # batches per core = 2
CT = C // 128              # 6 contraction chunks
TT = T // 128              # 8 t/s blocks of 128
NJ = T // 512              # 2 chunks of 512
SCALE = 1.0 / np.sqrt(H).astype(np.float32)

# wts column layout (all bf16): [wq 384 | wkv 768 | identity 128 |
#   negSL 128 | onesrow 128 | bvb 512 | bq,bk-as-bf16-bits 4]
WQ0, WKV0, ID0, SL0, ON0, BV0, BQ0 = 0, 384, 1152, 1280, 1408, 1536, 2048
WTSW = 2052


def _patched_drain_and_barrier(self, tick_clock, wait_clock):
    # This container's walrus build allows only ONE sync-wait command on a
    # CTRL-class (Drain) instruction; stock Tile attaches one wait per live
    # semaphore to a single tail drain. Split into a chain of drains.
    nc = self.nc
    drain_inst = nc.sync.drain()
    wait_clock.add_sem_waits(
        drain_inst.ins, ScopedClock({None: tick_clock.global_clock})
    )
    mi = drain_inst.ins
    si = mi.sync_info
    if si is not None and len(si.on_wait) > 1:
        waits = list(si.on_wait)
        mi.sync_info = mybir.SyncInfo(on_wait=waits[:1], on_update=list(si.on_update))
        for w in waits[1:]:
            d2 = nc.sync.drain()
            d2.ins.sync_info = mybir.SyncInfo(on_wait=[w], on_update=[])
    nc.all_engine_barrier()
    assert self.sems is not None
    popped = nc._tile_sem_poison_stack.pop()
    assert popped is self._sem_poison
    nc.clear_and_free_semaphores(list(self.sems.allocated().values()))
    nc.all_engine_barrier()


tile.TileContext._drain_and_barrier = _patched_drain_and_barrier


def _split_excess_waits(nc, max_waits=1):
    # Same walrus limitation for every instruction class: at most one
    # sync-wait command. Hoist extra waits onto standalone EventSemaphore
    # instructions placed immediately before, on the same engine.
    n_new = 0
    for f in nc.m.functions:
        for bb in f.blocks:
            new_insts = []
            for inst in bb.instructions:
                si = inst.sync_info
                if si is not None and len(si.on_wait) > max_waits:
                    waits = list(si.on_wait)
                    for k, w in enumerate(waits[max_waits:]):
                        ev = mybir.InstEventSemaphore(
                            name=f"{inst.name}-xw{k}", ins=[], outs=[]
                        )
                        ev.engine = inst.engine
                        ev.sync_info = mybir.SyncInfo(on_wait=[w], on_update=[])
                        new_insts.append(ev)
                        n_new += 1
                    inst.sync_info = mybir.SyncInfo(
                        on_wait=waits[:max_waits], on_update=list(si.on_update)
                    )
                new_insts.append(inst)
            bb.instructions = new_insts
    return n_new


def _build_nc(reps=1):
    nc = bass.Bass()
    xt = nc.declare_dram_parameter("xt", [BPC, C, T], bf16, isOutput=False)
    wts = nc.declare_dram_parameter("wts", [128, WTSW], bf16, isOutput=False)
    # output in transposed layout [H, T] per batch; host transposes back
    out = nc.declare_dram_parameter("out", [BPC, H, T], f32, isOutput=True)

    with ExitStack() as ctx:
        tc = ctx.enter_context(tile.TileContext(nc))
        const = ctx.enter_context(tc.tile_pool(name="const", bufs=1))
        xt_pool = ctx.enter_context(tc.tile_pool(name="xt_pool", bufs=2 * CT))
        qk_pool = ctx.enter_context(tc.tile_pool(name="qk_pool", bufs=4))
        vthi_pool = ctx.enter_context(tc.tile_pool(name="vthi_pool", bufs=2))
        vaug_pool = ctx.enter_context(tc.tile_pool(name="vaug_pool", bufs=2))
        et_pool = ctx.enter_context(tc.tile_pool(name="et_pool", bufs=4))
        outT_pool = ctx.enter_context(tc.tile_pool(name="outT_pool", bufs=2))
        rec_pool = ctx.enter_context(tc.tile_pool(name="rec_pool", bufs=2))
        ps_proj = ctx.enter_context(tc.tile_pool(name="ps_proj", bufs=2, space="PSUM"))
        ps_st = ctx.enter_context(tc.tile_pool(name="ps_st", bufs=3, space="PSUM"))
        ps_out = ctx.enter_context(tc.tile_pool(name="ps_out", bufs=2, space="PSUM"))
        ps_aux = ctx.enter_context(tc.tile_pool(name="ps_aux", bufs=1, space="PSUM"))

        wts_sb = const.tile([128, WTSW], bf16)
        nc.sync.dma_start(wts_sb[:, :], wts[:, :])
        bq_ap = wts_sb[0:64, BQ0 : BQ0 + 2].bitcast(f32)
        bk_ap = wts_sb[0:64, BQ0 + 2 : BQ0 + 4].bitcast(f32)

        def proj_phase(rep, b):
                u = f"{rep}_{b}"
                xts = []
                for c in range(CT):
                    xt_c = xt_pool.tile([128, T], bf16, name=f"xt_{u}_{c}", tag="xt")
                    nc.sync.dma_start(xt_c[:, :], xt[b, 128 * c : 128 * (c + 1), :])
                    xts.append(xt_c)

                q_sb = qk_pool.tile([64, T], bf16, name=f"q_{u}", tag="q")
                k_sb = qk_pool.tile([64, T], bf16, name=f"k_{u}", tag="k")
                vthi = vthi_pool.tile([128, T], bf16, name=f"vthi_{u}", tag="vthi")

                for n in range(NJ):
                    ncol = slice(512 * n, 512 * (n + 1))
                    q_ps = ps_proj.tile(
                        [64, 512], f32, name=f"qps_{u}_{n}", tag="ps_proj"
                    )
                    for c in range(CT):
                        nc.tensor.matmul(
                            q_ps[:, :],
                            lhsT=wts_sb[:, WQ0 + 64 * c : WQ0 + 64 * (c + 1)],
                            rhs=xts[c][:, ncol],
                            start=(c == 0),
                            stop=(c == CT - 1),
                        )
                    nc.vector.tensor_scalar_add(q_sb[:, ncol], q_ps[:, :], bq_ap)

                    kv_ps = ps_proj.tile(
                        [128, 512], f32, name=f"kvps_{u}_{n}", tag="ps_proj"
                    )
                    for c in range(CT):
                        nc.tensor.matmul(
                            kv_ps[:, :],
                            lhsT=wts_sb[:, WKV0 + 128 * c : WKV0 + 128 * (c + 1)],
                            rhs=xts[c][:, ncol],
                            start=(c == 0),
                            stop=(c == CT - 1),
                        )
                    nc.vector.tensor_scalar_add(
                        k_sb[:, ncol], kv_ps[0:64, :], bk_ap
                    )
                    nc.scalar.copy(vthi[64:128, ncol], kv_ps[64:128, :])

                # v into [s, h|1|0] augmented layout: 8 PE transposes into one
                # half-bank bf16 psum tile, then a single strided DVE add
                # (folds the v-bias broadcast tile) + ones/zeros memsets.
                # Column 64 of each 128-block = ones, so the out matmul's
                # partition 64 accumulates the softmax denominator.
                vtr_ps = ps_aux.tile(
                    [128, 512], bf16, name=f"vtr_{u}", tag="ps_aux"
                )
                for si in range(TT):
                    nc.tensor.transpose(
                        vtr_ps[:, 64 * si : 64 * (si + 1)],
                        vthi[64:128, 128 * si : 128 * (si + 1)],
                        wts_sb[64:128, ID0 + 64 : ID0 + 128],
                    )
                vaug = vaug_pool.tile([128, 1024], bf16, name=f"va_{u}", tag="vaug")
                va3 = vaug[:, :].rearrange("p (g c) -> p g c", c=128)
                nc.vector.tensor_add(
                    va3[:, :, 0:64],
                    vtr_ps[:, :].rearrange("p (g c) -> p g c", c=64),
                    wts_sb[:, BV0 : BV0 + 512].rearrange("p (g c) -> p g c", c=64),
                )
                # all aug columns ones: out_ps partitions 64..127 each get the
                # softmax denominator, so normalization needs no broadcast.
                nc.vector.memset(va3[:, :, 64:128], 1.0)
                return q_sb, k_sb, vaug

        def attn_phase(rep, b, q_sb, k_sb, vaug):
                u = f"{rep}_{b}"
                outT = outT_pool.tile([64, T], f32, name=f"outT_{u}", tag="outT")
                ILAST = {0: 3, 1: 7}

                def normalize(j, out_ps):
                    # partitions 64..127 of out_ps all hold the denominator
                    # (aug ones columns); reciprocal + mixed-base multiply on
                    # DVE, no broadcast needed.
                    rec = rec_pool.tile(
                        [128, 512], f32, name=f"rec_{u}_{j}", tag="rec"
                    )
                    nc.vector.reciprocal(rec[64:128, :], out_ps[64:128, :])
                    nc.vector.tensor_mul(
                        outT[:, 512 * j : 512 * (j + 1)],
                        out_ps[0:64, :],
                        rec[64:128, :],
                    )

                # i-outer: one k/vaug weight load per s-block serves both
                # column chunks. Out matmuls are software-pipelined one block
                # behind (the in-order PE queue must not stall on exp(i)).
                out_tiles = {
                    j: ps_out.tile(
                        [128, 512], f32, name=f"ops_{u}_{j}", tag="ps_out"
                    )
                    for j in range(NJ)
                }
                pend = None  # (i, [(j, cc)], {j: et}) awaiting out matmuls

                def flush(pend):
                    pi, pchunks, pets = pend
                    for j, cc in pchunks:
                        nc.tensor.matmul(
                            out_tiles[j][:, cc:512],
                            lhsT=vaug[:, 128 * pi : 128 * (pi + 1)],
                            rhs=pets[j][:, cc:512],
                            start=(pi == 0),
                            stop=(pi == ILAST[j]),
                            skip_group_check=True,
                        )
                        if pi == ILAST[j]:
                            normalize(j, out_tiles[j])

                for i in range(TT):
                    chunks = [
                        (j, max(128 * i - 512 * j, 0))
                        for j in range(NJ)
                        if 128 * i < 512 * (j + 1)
                    ]
                    sts = {}
                    for j, cc in chunks:
                        st_ps = ps_st.tile(
                            [128, 512], f32, name=f"st_{u}_{i}_{j}", tag="ps_st"
                        )
                        nc.tensor.matmul(
                            st_ps[:, cc:512],
                            lhsT=k_sb[:, 128 * i : 128 * (i + 1)],
                            rhs=q_sb[:, 512 * j + cc : 512 * (j + 1)],
                            start=True,
                            stop=True,
                        )
                        sts[j] = st_ps
                    ets = {}
                    for j, cc in chunks:
                        et = et_pool.tile(
                            [128, 512], bf16, name=f"et_{u}_{i}_{j}", tag="et"
                        )
                        nc.scalar.activation(
                            et[:, cc:512], sts[j][:, cc:512], AF.Exp,
                            scale=float(SCALE),
                        )
                        ets[j] = et
                    # causal mask on the diagonal block: bf16 upper-tri
                    # multiply on DVE (2x 16-bit rate)
                    jstar, dcc = i // 4, 128 * (i % 4)
                    nc.vector.tensor_mul(
                        ets[jstar][:, dcc : dcc + 128],
                        ets[jstar][:, dcc : dcc + 128],
                        wts_sb[:, SL0 : SL0 + 128],
                    )
                    if pend is not None:
                        flush(pend)
                    pend = (i, chunks, ets)
                flush(pend)

                nc.sync.dma_start(out[b], outT[:, :])

        # phase-split emission: both batches' projection work is queued
        # before either batch's attention, so the in-order PE queue can fill
        # attention-phase stalls with the other batch's projection matmuls.
        for rep in range(reps):
            state = [proj_phase(rep, b) for b in range(BPC)]
            for b in range(BPC):
                attn_phase(rep, b, *state[b])
    _split_excess_waits(nc)
    return nc


_NC_CACHE = None


def _get_nc():
    global _NC_CACHE
    if _NC_CACHE is None:
        _NC_CACHE = _build_nc()
    return _NC_CACHE


def _prep_in_maps(x, Wq, bq, Wk, bk, Wv, bv):
    import ml_dtypes

    bf = ml_dtypes.bfloat16
    x = np.asarray(x, dtype=np.float32)
    Wq = np.asarray(Wq, dtype=np.float32)
    Wk = np.asarray(Wk, dtype=np.float32)
    Wv = np.asarray(Wv, dtype=np.float32)
    bq = np.asarray(bq, dtype=np.float32)
    bk = np.asarray(bk, dtype=np.float32)
    bv = np.asarray(bv, dtype=np.float32)

    wts = np.zeros((128, WTSW), dtype=bf)
    for c in range(CT):
        wts[:, WQ0 + 64 * c : WQ0 + 64 * (c + 1)] = Wq[128 * c : 128 * (c + 1)]
        wts[:, WKV0 + 128 * c : WKV0 + 128 * c + 64] = Wk[128 * c : 128 * (c + 1)]
        wts[:, WKV0 + 128 * c + 64 : WKV0 + 128 * (c + 1)] = Wv[
            128 * c : 128 * (c + 1)
        ]
    wts[:, ID0 : ID0 + 128] = np.eye(128, dtype=np.float32)
    # SL0 block: upper-triangular-inclusive ones = causal keep-mask for the
    # diagonal [s, t] block (valid where s <= t)
    wts[:, SL0 : SL0 + 128] = np.triu(np.ones((128, 128), dtype=np.float32))
    wts[64, ON0 : ON0 + 128] = 1.0
    wts[:, BV0 : BV0 + 512] = np.tile(bv, (128, 8))
    wts[0:64, BQ0 : BQ0 + 2] = bq.reshape(64, 1).astype("<f4").view(np.uint16).view(bf)
    wts[0:64, BQ0 + 2 : BQ0 + 4] = (
        bk.reshape(64, 1).astype("<f4").view(np.uint16).view(bf)
    )

    in_maps = []
    for i in range(NCORES):
        xs = np.ascontiguousarray(
            x[BPC * i : BPC * (i + 1)].transpose(0, 2, 1)
        ).astype(bf)  # [BPC, C, T]
        in_maps.append({"xt": xs, "wts": wts})
    return in_maps


def run(inputs, trace=False, **spmd_kwargs):
    from concourse.bass_utils import run_bass_kernel_spmd

    nc = _get_nc()
    in_maps = _prep_in_maps(**inputs)
    res = run_bass_kernel_spmd(
        nc, in_maps, list(range(NCORES)), trace=trace, **spmd_kwargs
    )
    out = np.concatenate([res.results[i]["out"] for i in range(NCORES)], axis=0)
    # device produced [B, H, T]; back to [B, T, H]
    out = np.ascontiguousarray(out.transpose(0, 2, 1))
    return out.astype(np.float32, copy=False), res


def kernel(**inputs) -> np.ndarray:
    out, _ = run(inputs)
    return out
